# revision 29
# baseline (speedup 1.0000x reference)
"""Trainium2 Bass kernel for nn_MinimalAttnHead.

Computes, per batch b:
    EW      = E @ W.T                       # [S, D]
    scores  = (E @ EW.T) / sqrt(D)          # [S, S]
    attn    = softmax(causal_mask(scores))  # causal: key k > query q masked
    out     = attn @ E                      # [S, D]

with B=4, S=4096, D=256, fp32.

Sharding: 8 cores = (batch b in 0..3) x (half h in 0..1). Queries of each
batch are split into 8 strips of 512; core half h=0 takes strips
{0,3,4,7}, h=1 takes {1,2,5,6} — causal work is exactly balanced.
Every core runs the SAME program; per-core data (query/key slices,
per-key additive bias) encodes which strips it owns.

Algorithm per core: since scores = E W E^T, the weight is applied on the
QUERY side once: XQ[d, q] = sum_e W[e, d] * E[q, e] over the core's 2048
queries.  For each (query strip, 128-key tile): S_T[k, q] =
sum_d ET[d, k] * XQ[d, q] — the stationary operand is the raw transposed
encodings, so everything but XQ streams straight from HBM.

Per strip (slot s = core's strips sorted ascending) the program processes
4 "diag" key-tiles (the strip's own keys, static intra-strip causal mask,
trimmed to the un-masked query range) plus PAST[s] = {4,12,20,28} "past"
key-tiles from keys[0 : 128*PAST[s]].  A per-key bias (0 or -1e6, added
inside the ACT exp) kills key-tiles a core doesn't need, keeping trip
counts uniform across cores.

Softmax is max-free (scores/16 ~ N(0,1), exp cannot overflow):
P_T = exp(S_T/16 + mask); numerator and denominator accumulate together
in PSUM via a ones-column appended to V; one reciprocal+scale per 128
queries normalizes.  Matmuls run in float32r (fp32 data, reduced-precision
PE mode — full rate at free-dim >= 256, ~1e-3 accuracy; plain fp32 is 4x
slower).

All DRAM tensors are laid out host-side in SBUF-native [128, free] form,
so every DMA is a contiguous-run 2D/3D copy; loads are split and ordered
by first-use time across the two HWDGE trigger queues (Sync + Scalar).
"""

import contextlib
import ctypes
import sys
import types

import numpy as np

for _p in ("/opt/trn_rl_repo",):
    if _p not in sys.path:
        sys.path.insert(0, _p)

import concourse.bacc as bacc
import concourse.bass as bass
import concourse.mybir as mybir
import concourse.tile as tile
from concourse import bass_utils

# ---------------------------------------------------------------- constants
B, S, D = 4, 4096, 256
QSTRIP = 512                    # queries per strip
NSLOT = 4                       # strips per core
PAST = [4, 12, 20, 28]          # past k-tiles (of 128 keys) per slot
PAST_KEYS = 128 * PAST[-1]      # 3584: keys ever read as "past"
STRIPS = {0: [0, 3, 4, 7], 1: [1, 2, 5, 6]}
NEG = -1.0e6                    # additive mask / bias value (exp -> 0)
INV_SQRT_D = 1.0 / 16.0
NQ = NSLOT * QSTRIP             # 2048 queries per core
VW = D + 2                      # V row: D cols + ones + zero pad (even free dim)
NPT = PAST_KEYS // 128          # 28 past key tiles
NDT = NQ // 128                 # 16 diag key tiles
# diag tile j covers local query cols [DIAG_LO[j]*128 : 512]; its triangular
# 128-block sits at local col (j - DIAG_LO[j])*128.  j=3 keeps width 256
# (fp32r needs moving free-dim >= 256), masking the dead u=2 block.
DIAG_LO = [0, 1, 2, 2]

F32 = mybir.dt.float32
F32R = mybir.dt.float32r
BF16 = mybir.dt.bfloat16

_CACHE = {}


# ------------------------------------------------------- axon NTFF trace shim
def _install_ntff_hook():
    """Provide antenv.axon_hooks (absent in this container) so
    run_bass_kernel_spmd(trace=True) can profile via libaxon_pjrt.so."""
    if "antenv.axon_hooks" in sys.modules:
        return
    try:
        import antenv
    except ImportError:
        return
    mod = types.ModuleType("antenv.axon_hooks")
    mod._hook = None
    mod.set_axon_ntff_profile_hook = lambda h: setattr(mod, "_hook", h)
    mod.get_axon_ntff_profile_hook = lambda: mod._hook
    sys.modules["antenv.axon_hooks"] = mod
    antenv.axon_hooks = mod
    try:
        lib = ctypes.CDLL("/opt/axon/libaxon_pjrt.so")
        lib.axon_start_nrt_profile.argtypes = [
            ctypes.POINTER(ctypes.c_int64),
            ctypes.c_size_t,
        ]
        lib.axon_start_nrt_profile.restype = ctypes.c_int64
        lib.axon_stop_nrt_profile.argtypes = [ctypes.c_char_p]
        lib.axon_stop_nrt_profile.restype = ctypes.c_int64
    except OSError:
        return

    @contextlib.contextmanager
    def _hook(output_dir, device_ids):
        import jax

        jax.devices()
        if device_ids:
            ids = (ctypes.c_int64 * len(device_ids))(*device_ids)
            rc = lib.axon_start_nrt_profile(ids, len(device_ids))
        else:
            rc = lib.axon_start_nrt_profile(None, 0)
        if rc != 0:
            raise RuntimeError(f"axon_start_nrt_profile rc={rc}")
        try:
            yield
        finally:
            lib.axon_stop_nrt_profile(str(output_dir).encode())

    mod._hook = _hook
    # artifact upload needs monorepo fish paths; keep traces local
    bass_utils.upload_artifacts = lambda tmpdir: "local://" + tmpdir


# ------------------------------------------------------------- program build
def _build():
    nc = bacc.Bacc("TRN2", target_bir_lowering=False, debug=False)

    # all inputs already in SBUF-native [128, free] layout (host packs them)
    etq_d = nc.dram_tensor("ETQ", [128, 2 * NQ], F32, kind="ExternalInput")
    etp_d = nc.dram_tensor("ETP", [128, 2 * PAST_KEYS], F32, kind="ExternalInput")
    vp_d = nc.dram_tensor("VP", [128, NPT * VW], BF16, kind="ExternalInput")
    vd_d = nc.dram_tensor("VD", [128, NDT * VW], BF16, kind="ExternalInput")
    w_d = nc.dram_tensor("W", [128, 2 * D], F32, kind="ExternalInput")
    mask_d = nc.dram_tensor("MASK", [128, 2 * 128], F32, kind="ExternalInput")
    bias_d = nc.dram_tensor("BIAS", [128, NSLOT * PAST[-1]], F32, kind="ExternalInput")
    out_d = nc.dram_tensor("OUT", [128, NDT * D], F32, kind="ExternalOutput")

    with tile.TileContext(nc) as tc:
        with (
            tc.tile_pool(name="persist", bufs=1) as pp,
            tc.tile_pool(name="psA", bufs=4, space=bass.MemorySpace.PSUM) as psA,
            tc.tile_pool(name="psO", bufs=1, space=bass.MemorySpace.PSUM) as psO,
            tc.tile_pool(name="work", bufs=8) as wp,
        ):
            # ---------------- persistent SBUF ----------------
            w_sb = pp.tile([128, 2 * D], F32R, tag="w", name="w")
            etq_sb = pp.tile([128, 2 * NQ], F32R, tag="etq", name="etq")
            etp_sb = pp.tile([128, 2 * PAST_KEYS], F32R, tag="etp", name="etp")
            xq_sb = pp.tile([128, 2 * NQ], F32R, tag="xq", name="xq")
            vp_sb = pp.tile([128, NPT * VW], BF16, tag="vp", name="vp")
            vd_sb = pp.tile([128, NDT * VW], BF16, tag="vd", name="vd")
            mask_sb = pp.tile([128, 2 * 128], F32, tag="mask", name="mask")
            bias_sb = pp.tile([128, NSLOT * PAST[-1]], F32, tag="bias", name="bias")

            def eth(t, h, c0, c1, n):
                return t[:, h * n + c0 : h * n + c1]

            def load_cols(dst, src_t, c0, c1, engine=None):
                (engine or nc.sync).dma_start(
                    dst[:, c0:c1], src_t.ap()[:, c0:c1]
                )

            def load_2h(dst, src_t, n, c0, c1, engine=None):
                (engine or nc.sync).dma_start(
                    dst[:].rearrange("p (h n) -> p h n", h=2)[:, :, c0:c1],
                    src_t.ap()
                    .rearrange("p (h n) -> p h n", h=2)[:, :, c0:c1]
                    .bitcast(F32R),
                )

            # PE warm-up: dense dummy matmuls on a memset tile, issued
            # before any load lands, so the HAM clock throttle ramps to 8/8
            # before real work starts (otherwise ~20us run at half rate).
            wusrc = pp.tile([128, 512], BF16, tag="wusrc", name="wusrc")
            nc.gpsimd.memset(wusrc[:], 1.0)
            for _ in range(22):
                wu = psA.tile([128, 512], F32, tag="st", name="wu")
                nc.tensor.matmul(
                    wu[:], wusrc[:, 0:128], wusrc[:], start=True, stop=True,
                )

            # -------- loads, ordered by first-use; two trigger queues ------
            nc.sync.dma_start(w_sb[:], w_d.ap().bitcast(F32R))   # Sync
            nc.scalar.dma_start(bias_sb[:], bias_d.ap())         # Scalar
            load_2h(etq_sb, etq_d, NQ, 0, QSTRIP)                # Sync: strip 0
            nc.scalar.dma_start(mask_sb[:], mask_d.ap())         # Scalar
            load_cols(vd_sb, vd_d, 0, 4 * VW, nc.scalar)         # diag V slot 0
            load_2h(etp_sb, etp_d, PAST_KEYS, 0, QSTRIP)         # Sync: past c0
            load_cols(vp_sb, vp_d, 0, 4 * VW, nc.scalar)         # past V slot 0
            load_2h(etq_sb, etq_d, NQ, QSTRIP, 2 * QSTRIP)       # Sync: strip 1
            load_cols(vd_sb, vd_d, 4 * VW, 8 * VW, nc.scalar)    # diag V slot 1
            load_2h(etp_sb, etp_d, PAST_KEYS, QSTRIP, 1536)      # slot 1 keys
            load_cols(vp_sb, vp_d, 4 * VW, 12 * VW, nc.scalar)
            load_2h(etq_sb, etq_d, NQ, 2 * QSTRIP, 3 * QSTRIP)   # Sync: strip 2
            load_cols(vd_sb, vd_d, 8 * VW, 12 * VW, nc.scalar)   # diag V slot 2
            load_2h(etp_sb, etp_d, PAST_KEYS, 1536, 2560)        # slot 2 keys
            load_cols(vp_sb, vp_d, 12 * VW, 20 * VW, nc.scalar)
            load_2h(etq_sb, etq_d, NQ, 3 * QSTRIP, NQ)           # Sync: strip 3
            load_cols(vd_sb, vd_d, 12 * VW, NDT * VW, nc.scalar) # diag V slot 3
            load_2h(etp_sb, etp_d, PAST_KEYS, 2560, PAST_KEYS)   # slot 3 keys
            load_cols(vp_sb, vp_d, 20 * VW, NPT * VW, nc.scalar)

            # XQ[d, q] = sum_e W[e, d] * ETQ[e, q], one 512-query chunk
            def xq_chunk(s):
                q0 = s * QSTRIP
                for dh in range(2):
                    ps = psA.tile([128, QSTRIP], F32, tag="st", name="st")
                    nc.tensor.matmul(
                        ps[:],
                        eth(w_sb, 0, dh * 128, (dh + 1) * 128, D),
                        eth(etq_sb, 0, q0, q0 + QSTRIP, NQ),
                        start=True,
                        stop=False,
                    )
                    nc.tensor.matmul(
                        ps[:],
                        eth(w_sb, 1, dh * 128, (dh + 1) * 128, D),
                        eth(etq_sb, 1, q0, q0 + QSTRIP, NQ),
                        start=False,
                        stop=True,
                    )
                    nc.vector.tensor_copy(
                        eth(xq_sb, dh, q0, q0 + QSTRIP, NQ), ps[:]
                    )

            # ---------------- attention ----------------
            xq_chunk(0)
            for s in range(NSLOT):
                q0 = s * QSTRIP
                outp = [
                    psO.tile([128, VW], F32, tag=f"outp{u}", name=f"outp{u}")
                    for u in range(4)
                ]
                nkt = 4 + PAST[s]
                if s == NSLOT - 1:
                    # diagonals LAST: outp[u] completes at diag u, so the
                    # norm/store epilogue overlaps the remaining diag matmuls
                    korder = [(False, jp) for jp in range(PAST[s])] + [
                        (True, t) for t in range(4)
                    ]
                else:
                    korder = [(True, t) for t in range(4)] + [
                        (False, jp) for jp in range(PAST[s])
                    ]
                for j, (diag, kt) in enumerate(korder):
                    lo = DIAG_LO[kt] if diag else 0     # first live q-subtile
                    nw = QSTRIP - lo * 128              # moving width
                    if diag:
                        kcol = (s * 4 + kt) * 128
                        et_src, et_n = etq_sb, NQ
                        vt = vd_sb[:, (s * 4 + kt) * VW : (s * 4 + kt + 1) * VW]
                    else:
                        kcol = kt * 128
                        et_src, et_n = etp_sb, PAST_KEYS
                        vt = vp_sb[:, kt * VW : (kt + 1) * VW]

                    st = psA.tile([128, nw], F32, tag="st", name="st")
                    nc.tensor.matmul(
                        st[:],
                        eth(et_src, 0, kcol, kcol + 128, et_n),
                        eth(xq_sb, 0, q0 + lo * 128, q0 + QSTRIP, NQ),
                        start=True,
                        stop=False,
                    )
                    nc.tensor.matmul(
                        st[:],
                        eth(et_src, 1, kcol, kcol + 128, et_n),
                        eth(xq_sb, 1, q0 + lo * 128, q0 + QSTRIP, NQ),
                        start=False,
                        stop=True,
                    )

                    p_t = wp.tile([128, nw], BF16, tag="P", name="pt")
                    if diag:
                        if kt < 3:
                            # triangular mask on the leading 128-block
                            nc.vector.tensor_add(
                                st[:, 0:128], st[:, 0:128], mask_sb[:, 128:256]
                            )
                        else:
                            # dead u=2 block (all -inf) + triangular u=3
                            nc.vector.tensor_add(
                                st[:, 0:256], st[:, 0:256], mask_sb[:, 0:256]
                            )
                        nc.scalar.activation(
                            p_t[:], st[:],
                            mybir.ActivationFunctionType.Exp,
                            scale=INV_SQRT_D,
                        )
                    else:
                        col = s * PAST[-1] + kt
                        nc.scalar.activation(
                            p_t[:], st[:],
                            mybir.ActivationFunctionType.Exp,
                            bias=bias_sb[:, col : col + 1],
                            scale=INV_SQRT_D,
                        )

                    ufirst = kt if diag else 0          # skip dead out-mms
                    for u in range(ufirst, 4):
                        if s == NSLOT - 1:
                            ustop = diag and kt == u
                        else:
                            ustop = j == nkt - 1
                        nc.tensor.matmul(
                            outp[u][:],
                            p_t[:, (u - lo) * 128 : (u - lo + 1) * 128],
                            vt,
                            start=(j == 0),
                            stop=ustop,
                        )

                    # overlap next slot's XQ with this slot's k-loop middle
                    if s + 1 < NSLOT and j == (4, 8, 12, 12)[s]:
                        xq_chunk(s + 1)

                res = wp.tile([128, 4 * D], F32, tag="res", name="res", bufs=2)
                for u in range(4):
                    den = wp.tile([128, 1], F32, tag="den", name="den")
                    nc.vector.reciprocal(den[:], outp[u][:, D : D + 1])
                    nc.vector.tensor_scalar_mul(
                        res[:, u * D : (u + 1) * D], outp[u][:, 0:D], den[:]
                    )
                    if s == NSLOT - 1:
                        # tail slot: store each 128-query group as soon as it
                        # normalizes (accumulators finish staggered there)
                        nc.sync.dma_start(
                            out_d.ap()[:, (s * 4 + u) * D : (s * 4 + u + 1) * D],
                            res[:, u * D : (u + 1) * D],
                        )
                if s != NSLOT - 1:
                    nc.sync.dma_start(
                        out_d.ap()[:, s * 4 * D : (s + 1) * 4 * D], res[:]
                    )

    nc.compile()
    return nc


def _get_program():
    if "nc" not in _CACHE:
        _CACHE["nc"] = _build()
    return _CACHE["nc"]


# ------------------------------------------------------------- host-side data
def _hw2(x):
    """[256, N] -> SBUF-native [128, 2N] (halves side by side)."""
    return np.concatenate([x[0:128], x[128:256]], axis=1)


def _hwtiles(x):
    """[(n*128), v] -> SBUF-native [128, n*v]."""
    n = x.shape[0] // 128
    return np.ascontiguousarray(
        x.reshape(n, 128, x.shape[1]).transpose(1, 0, 2).reshape(128, -1)
    )


def _static_inputs():
    """MASK / BIAS-per-half: identical across calls."""
    if "static" in _CACHE:
        return _CACHE["static"]
    tri = np.where(
        np.arange(128)[:, None] <= np.arange(128)[None, :], 0.0, NEG
    ).astype(np.float32)
    # mask layout: [block of NEG | tri]; j<3 uses cols 128:256, j=3 uses 0:256
    mask = np.concatenate(
        [np.full((128, 128), NEG, np.float32), tri], axis=1
    )
    biases = {}
    for h in (0, 1):
        bias = np.zeros((128, NSLOT * PAST[-1]), np.float32)
        for s, p in enumerate(sorted(STRIPS[h])):
            for kt in range(PAST[s]):
                keys = kt * 128 + np.arange(128)
                col = s * PAST[-1] + kt
                bias[:, col] = np.where(keys < QSTRIP * p, 0.0, NEG)
        biases[h] = bias
    _CACHE["static"] = (mask, biases)
    return _CACHE["static"]


def _core_inputs(encodings, W):
    mask, biases = _static_inputs()
    w = _hw2(np.ascontiguousarray(np.asarray(W, np.float32)))
    ones = np.ones((1,), np.float32)
    pad = np.zeros((1,), np.float32)
    in_maps = []
    for c in range(8):
        b, h = c // 2, c % 2
        e = np.asarray(encodings[b], np.float32)          # [S, D]
        et = np.ascontiguousarray(e.T)                    # [D, S]
        strips = sorted(STRIPS[h])
        etq = np.concatenate(
            [et[:, p * QSTRIP : (p + 1) * QSTRIP] for p in strips], axis=1
        )
        rows_d = np.concatenate(
            [e[p * QSTRIP : (p + 1) * QSTRIP] for p in strips], axis=0
        )
        vd = np.concatenate(
            [rows_d, np.broadcast_to(ones, (NQ, 1)),
             np.broadcast_to(pad, (NQ, 1))], axis=1
        )
        vp = np.concatenate(
            [e[:PAST_KEYS], np.broadcast_to(ones, (PAST_KEYS, 1)),
             np.broadcast_to(pad, (PAST_KEYS, 1))], axis=1
        )
        bf16 = mybir.dt.np(BF16)
        in_maps.append(
            {
                "ETQ": np.ascontiguousarray(_hw2(etq)),
                "ETP": np.ascontiguousarray(_hw2(et[:, :PAST_KEYS])),
                "VP": _hwtiles(vp).astype(bf16),
                "VD": _hwtiles(vd).astype(bf16),
                "W": np.ascontiguousarray(w),
                "MASK": mask,
                "BIAS": biases[h],
            }
        )
    return in_maps


def run_on_cores(encodings, W, trace=False, trace_cores=None):
    """Build+run; returns (output [B,S,D], BassKernelResults)."""
    _install_ntff_hook()
    nc = _get_program()
    in_maps = _core_inputs(encodings, W)
    res = bass_utils.run_bass_kernel_spmd(
        nc,
        in_maps,
        core_ids=list(range(8)),
        trace=trace,
        trace_cores=trace_cores,
    )
    out = np.empty((B, S, D), np.float32)
    for c in range(8):
        b, h = c // 2, c % 2
        o = res.results[c]["OUT"]                          # [128, 16*256]
        o = o.reshape(128, NDT, D).transpose(1, 0, 2).reshape(NQ, D)
        for s, p in enumerate(sorted(STRIPS[h])):
            out[b, p * QSTRIP : (p + 1) * QSTRIP, :] = o[
                s * QSTRIP : (s + 1) * QSTRIP
            ]
    return out, res


def kernel(encodings, W):
    out, _ = run_on_cores(encodings, W, trace=False)
    return out
